# revision 35
# baseline (speedup 1.0000x reference)
"""Trainium2 Bass kernel for nn_Attention_50027779064227.

Computes softmax(v . tanh([hidden, enc] @ W + b)) over the source axis.
Data-parallel over batch across 8 NeuronCores; W/b/v replicated.

Algebraic split: concat([hid, enc]) @ W = hidden @ W_h (tiny -> computed
on HOST, shipped as a 16KB per-partition bias table) + enc @ W_e (the
big matmul, fp16 operands at full TensorE rate, fp32 PSUM accumulation).
The host-side h-part plus bias b is folded into the ScalarE tanh
activation as a per-partition bias. The v-dot (cross-partition
reduction) is an INCREMENTAL VectorE fold (one step per tanh d-block as
it lands) plus one ones-vector matmul per chunk; per-batch softmax runs
inline, with exp reading the score rows directly from PSUM (no DVE
copy). No max-subtraction: |scores| < 30 here, fp32 exp is safe.

Startup choreography (the kernel is PE-stream-bound, so everything else
hides behind the matmul stream except the first ~0.25MB of DMA):
- the first FOUR chunks are processed K-MAJOR from k-granular 128KB
  DMA pieces (2KB->1KB lines), so the first real matmul needs only
  one 128KB weight piece + one 128KB enc piece;
- weights are stored fully k-major ([KT, P, DT*P]) and arrive as 8
  k-granular pieces interleaved with chunk0's enc pieces across the
  two HWDGE rings in consumption order;
- a short warmup burst (M=1 matmuls into the score PSUM bank) covers
  the remaining DMA window so the HAM clock-gate opens to 2.4GHz
  roughly when real work begins.
"""
import sys

for _p in ("/opt/trn_rl_repo",):
    if _p not in sys.path:
        sys.path.insert(0, _p)

import numpy as np
import concourse.bass as bass
import concourse.bacc as bacc
import concourse.mybir as mybir
from concourse.tile import TileContext
from concourse.bass_utils import run_bass_kernel_spmd

P = 128
NCORES = 8
B, S, DK, DD = 64, 1024, 1024, 512  # batch, src len, 2*ENC_HID, DEC_HID
BL = B // NCORES                    # 8 batches per core
SW = 512                            # moving-dim tile (s columns per matmul)
SBLK = S // SW                      # 2 s-blocks
KT = DK // P                        # 8 k-tiles for W_e
DT = DD // P                        # 4 d-blocks
NKM = 4                             # chunks processed k-major at startup
NCH = BL * SBLK                     # 16 chunks per core
NWARM = 15
SMC = DT * BL + DT + 1              # smalls cols: hpre | v | ones

F32 = mybir.dt.float32
F32R = mybir.dt.float32r
F16 = mybir.dt.float16
TANH = mybir.ActivationFunctionType.Tanh
EXP = mybir.ActivationFunctionType.Exp
IDENT = mybir.ActivationFunctionType.Identity

_BUILT = None


def _build():
    nc = bacc.Bacc()
    # chunk0/weights arrive as 128KB k-singles (k-pair experiments put too
    # many bytes in flight and delayed the FIRST piece by ~3us -> HAM cold
    # restart); chunks 1-3 as 256KB k-pairs
    ekm_d = nc.declare_dram_parameter("ekm", [KT, P, SW], F16, isOutput=False)
    ekmp_d = nc.declare_dram_parameter("ekmp", [NKM, KT // 2, P, 2 * SW],
                                       F16, isOutput=False)
    # whole-chunk tiles for the d-major steady chunks 4..15
    enc_d = nc.declare_dram_parameter("enc", [NCH - NKM, P, KT * SW], F16,
                                      isOutput=False)
    wek_d = nc.declare_dram_parameter("wek", [KT, P, DT * P], F16, isOutput=False)
    sm_d = nc.declare_dram_parameter("sm", [P, SMC], F32, isOutput=False)
    out_d = nc.declare_dram_parameter("out", [BL, S], F32, isOutput=True)

    with TileContext(nc) as tc:
        with (
            tc.tile_pool(name="const", bufs=1) as cpool,
            tc.tile_pool(name="chunk", bufs=4) as chpool,
            tc.tile_pool(name="tanh", bufs=12) as thpool,
            tc.tile_pool(name="ps_e", bufs=6, space="PSUM") as pe_pool,
            tc.tile_pool(name="ps_sc", bufs=2, space="PSUM") as sc_pool,
        ):
            # --- HAM warmup: M=1 dummy matmuls (into the score PSUM bank,
            # costing no extra banks) keep the PE busy through the startup
            # DMA window so the clock-gate opens to 2.4GHz early ---
            warm = cpool.tile([P, 256], F16, tag="warm")
            nc.vector.memset(warm[:], 0.25)
            wps = sc_pool.tile([1, 256], F32, tag="scp", name="warmps")
            for i in range(NWARM):
                nc.tensor.matmul(wps[:], warm[:, 0:1], warm[:],
                                 start=(i == 0), stop=(i == NWARM - 1))

            # --- startup DMAs. HWDGE ring emission costs ~0.65us PER
            # dma_start regardless of size, so the critical prefix is
            # chunk0's k-granular pieces (Act) and the weight pieces (SP)
            # ONLY; the small tables ride as one merged DMA after chunk0,
            # and chunks 1-3 use k-pair pieces to halve emission count ---
            wek_t = cpool.tile([P, KT * DT * P], F16, tag="wek")
            ekm_t = [cpool.tile([P, KT * SW], F16, tag=f"ekm{c}", name=f"ekm{c}")
                     for c in range(NKM)]
            sm = cpool.tile([P, SMC], F32, tag="sm")

            # (a third SWDGE ring was tried here and LOST ~8us: its packets
            # round-robin against the latency-critical chunk0/weight pieces
            # with no QoS, starving the PE and re-throttling the HAM clock)
            DP = DT * P
            for k in range(KT):
                nc.sync.dma_start(wek_t[:, k * DP:(k + 1) * DP], wek_d[k])
                nc.scalar.dma_start(ekm_t[0][:, k * SW:(k + 1) * SW], ekm_d[k])
            # chunk1 k-pairs split across BOTH rings (behind 8 weks on SP
            # alone they landed ~2us late); then smalls, chunk2 (Act),
            # chunk3 (SP)
            for j in range(KT // 2):
                eng = nc.sync if j % 2 == 0 else nc.scalar
                eng.dma_start(ekm_t[1][:, 2 * j * SW:(2 * j + 2) * SW],
                              ekmp_d[1, j])
            nc.scalar.dma_start(sm[:], sm_d[:])
            for j in range(KT // 2):
                nc.scalar.dma_start(ekm_t[2][:, 2 * j * SW:(2 * j + 2) * SW],
                                    ekmp_d[2, j])
            for j in range(KT // 2):
                nc.sync.dma_start(ekm_t[3][:, 2 * j * SW:(2 * j + 2) * SW],
                                  ekmp_d[3, j])

            ones_f = cpool.tile([P, 1], F32, tag="onesf")
            nc.vector.memset(ones_f[:], 1.0)
            ones_c = cpool.tile([P, 1], F32R, tag="ones")
            nc.vector.tensor_copy(ones_c[:], ones_f[:])
            ones_t = ones_c[:]

            def we_ap(k, d):
                return wek_t[:, k * DT * P + d * P: k * DT * P + (d + 1) * P]

            def hpre_ap(d, b):
                return sm[:, d * BL + b:d * BL + b + 1]

            v_sc = [sm[:, DT * BL + d:DT * BL + d + 1] for d in range(DT)]

            # chunk ci = (batch, sblock), batch-major
            chunks = [(b, sb) for b in range(BL) for sb in range(SBLK)]
            pre_ch = {}

            def emit_chunk_dma(ci):
                t = chpool.tile([P, KT * SW], F16, tag="chunk", name=f"ch{ci}")
                eng = nc.scalar if ci % 2 == 0 else nc.sync
                eng.dma_start(t[:], enc_d[ci - NKM])
                pre_ch[ci] = t

            # --- per-chunk state ---
            u_tiles = {}        # ci -> fold accumulator (ready after d3 step)
            sc_tiles = {}       # ci -> score PSUM tile [1, SW]
            ex_tiles = {}       # batch -> exp row [1, S]
            ss_tiles = {}       # batch -> running sum [1, 1]

            def emit_tanh_fold(ci, d, pe, sl=None):
                """tanh of PSUM d-block into SBUF, then one incremental
                DVE fold step into u[ci]. `sl` is the column sub-slice of
                the s-window (None = full width)."""
                b = chunks[ci][0]
                if sl is None:
                    sl = slice(0, SW)
                th = thpool.tile([P, sl.stop - sl.start], F32R,
                                 tag="tanh", name=f"th{ci}d{d}")
                nc.scalar.activation(th[:], pe, TANH, bias=hpre_ap(d, b))
                if d == 0 and sl.stop - sl.start == SW:
                    u = thpool.tile([P, SW], F32R, tag="u", name=f"u{ci}")
                    u_tiles[ci] = u
                    nc.vector.tensor_scalar_mul(u[:], th[:], v_sc[0])
                else:
                    u = u_tiles[ci]
                    nc.vector.scalar_tensor_tensor(
                        u[:, sl], th[:], v_sc[d], u[:, sl],
                        op0=mybir.AluOpType.mult, op1=mybir.AluOpType.add,
                    )

            def emit_score(ci):
                """ones-vector matmul folding partitions of u[ci] into a
                score row kept in PSUM (exp reads it there directly)."""
                scp = sc_pool.tile([1, SW], F32, tag="scp", name=f"sc{ci}")
                nc.tensor.matmul(scp[:], ones_t, u_tiles[ci][:],
                                 start=True, stop=True)
                sc_tiles[ci] = scp

            def emit_exps(b):
                """Both sblock scores of batch b are in PSUM: exp them into
                the row tile with accumulated sums."""
                ex = cpool.tile([1, S], F32, tag=f"ex{b}", name=f"ex{b}")
                s0 = cpool.tile([1, 1], F32, tag=f"ss{b}a", name=f"ss{b}a")
                s1 = cpool.tile([1, 1], F32, tag=f"ss{b}b", name=f"ss{b}b")
                nc.scalar.activation(ex[:, 0:SW], sc_tiles.pop(2 * b)[:],
                                     EXP, accum_out=s0[:])
                nc.scalar.activation(ex[:, SW:S], sc_tiles.pop(2 * b + 1)[:],
                                     EXP, accum_out=s1[:])
                nc.vector.tensor_add(s0[:], s0[:], s1[:])
                ex_tiles[b] = ex
                ss_tiles[b] = s0

            def emit_row_softmax(b):
                # scale on ScalarE so the DVE stays free for fold steps
                rc = cpool.tile([1, 1], F32, tag=f"rc{b}", name=f"rc{b}")
                nc.vector.reciprocal(rc[:], ss_tiles[b][:])
                ex = ex_tiles[b]
                nc.scalar.activation(ex[:], ex[:], IDENT, scale=rc[:])
                nc.sync.dma_start(out_d[b:b + 1, :], ex[:])

            # --- k-major startup chunks: each matmul needs exactly one
            # 128KB weight piece + one 128KB enc piece ---
            for ci in range(NKM):
                pes = [pe_pool.tile([P, SW], F32, tag="pe", name=f"pe{ci}d{d}")
                       for d in range(DT)]
                for k in range(KT):
                    src = ekm_t[ci][:, k * SW:(k + 1) * SW]
                    for d in range(DT):
                        nc.tensor.matmul(pes[d][:], we_ap(k, d), src,
                                         start=(k == 0), stop=(k == KT - 1))
                    if k == 2 and ci >= 1:
                        emit_score(ci - 1)
                    if k == 4 and ci >= 2 and ci % 2 == 0:
                        emit_exps(ci // 2 - 1)
                    if k == 6 and ci % 2 == 0 and ci >= 2:
                        emit_row_softmax(ci // 2 - 1)
                for d in range(DT):
                    emit_tanh_fold(ci, d, pes[d][:])

            # prefetch first d-major chunks while chunk2/3 mains run
            # (depth 3: a 1MB chunk's round-robin share of the congested
            # SDMA pipe is only ~70GB/s, so 2-deep arrived ~0.7us late)
            emit_chunk_dma(NKM)
            emit_chunk_dma(NKM + 1)
            emit_chunk_dma(NKM + 2)

            # --- steady d-major chunks ---
            for ci in range(NKM, NCH):
                b, sb = chunks[ci]
                ch = pre_ch.pop(ci)
                if ci + 3 < NCH:
                    emit_chunk_dma(ci + 3)
                last = ci == NCH - 1

                def d_mains(d, pe, cols=slice(0, SW)):
                    n = cols.stop - cols.start
                    for k in range(KT):
                        nc.tensor.matmul(
                            pe[:, 0:n], we_ap(k, d),
                            ch[:, k * SW + cols.start: k * SW + cols.stop],
                            start=(k == 0), stop=(k == KT - 1))

                if not last:
                    for d in range(DT):
                        pe = pe_pool.tile([P, SW], F32, tag="pe", name="pe")
                        d_mains(d, pe)
                        emit_tanh_fold(ci, d, pe[:])
                        if d == 0:
                            emit_score(ci - 1)
                        if d == 1 and ci % 2 == 0:
                            emit_exps(ci // 2 - 1)
                        if d == 2 and ci % 2 == 0:
                            emit_row_softmax(ci // 2 - 1)
                else:
                    # final chunk (b7, s1): d0-2 normal; d3 in column halves
                    # with separate PSUM tiles so each half's tanh/fold/score
                    # chain starts at its own stop
                    for d in range(DT - 1):
                        pe = pe_pool.tile([P, SW], F32, tag="pe", name="pe")
                        d_mains(d, pe)
                        emit_tanh_fold(ci, d, pe[:])
                        if d == 0:
                            emit_score(ci - 1)      # (b7, s0)
                        if d == 1:
                            # early exp of (b7, s0) straight from PSUM
                            exL = cpool.tile([1, S], F32, tag="exL", name="exL")
                            s0L = cpool.tile([1, 1], F32, tag="s0L", name="s0L")
                            nc.scalar.activation(exL[:, 0:SW],
                                                 sc_tiles.pop(2 * b)[:],
                                                 EXP, accum_out=s0L[:])
                    peh = [pe_pool.tile([P, SW], F32, tag="pe", name=f"peh{h}")
                           for h in (0, 1)]
                    for h in (0, 1):
                        d_mains(DT - 1, peh[h], slice(h * 256, (h + 1) * 256))
                    # half 0's chain hides under half 1's mains; half 1's
                    # post-stop chain runs at 128-col granularity so the
                    # very last tanh/fold/score/exp pipeline is shallower
                    sadd = []
                    pieces = [(peh[0], slice(0, 256)),
                              (peh[1], slice(256, 384)), (peh[1], slice(384, SW))]
                    for pi, (pet, sl) in enumerate(pieces):
                        n = sl.stop - sl.start
                        emit_tanh_fold(ci, DT - 1,
                                       pet[:, sl.start % 256:sl.start % 256 + n],
                                       sl=sl)
                        scp = sc_pool.tile([1, n], F32, tag="scp",
                                           name=f"scL{pi}")
                        nc.tensor.matmul(scp[:], ones_t, u_tiles[ci][:, sl],
                                         start=True, stop=True)
                        sh = cpool.tile([1, 1], F32, tag=f"sL{pi}",
                                        name=f"sL{pi}")
                        nc.scalar.activation(exL[:, SW + sl.start:SW + sl.stop],
                                             scp[:], EXP, accum_out=sh[:])
                        sadd.append(sh)
                    for sh in sadd:
                        nc.vector.tensor_add(s0L[:], s0L[:], sh[:])
                    rc = cpool.tile([1, 1], F32, tag="rcL", name="rcL")
                    nc.vector.reciprocal(rc[:], s0L[:])
                    nc.vector.tensor_scalar_mul(exL[:, 0:SW], exL[:, 0:SW],
                                                rc[:])
                    nc.scalar.activation(exL[:, SW:S], exL[:, SW:S],
                                         IDENT, scale=rc[:])
                    nc.sync.dma_start(out_d[b:b + 1, :], exL[:])

    nc.finalize()
    return nc


def _run_spmd(hidden, encoder_outputs, W, b, v, trace=False, tmpdir=None):
    global _BUILT
    if _BUILT is None:
        _BUILT = _build()
    nc = _BUILT

    hidden = np.asarray(hidden, dtype=np.float64)
    W64 = np.asarray(W, dtype=np.float64)
    bv = np.asarray(b, dtype=np.float64)
    vv = np.asarray(v, dtype=np.float32)

    # weights fully k-major: wek[k] = W_e rows [k*128, (k+1)*128)
    wek = np.ascontiguousarray(
        np.asarray(W, dtype=np.float32)[DD:].astype(np.float16)
    ).reshape(KT, P, DT * P)

    # host-side tiny part: hpre[b] = hidden[b] @ W_h + b  -> [B, DD]
    hpre = (hidden @ W64[:DD] + bv).astype(np.float32)

    encT = np.transpose(np.asarray(encoder_outputs, dtype=np.float32),
                        (1, 2, 0)).astype(np.float16)     # [B, DK, S]
    vr = vv.reshape(DT, P)

    in_maps = []
    for c in range(NCORES):
        shard = encT[c * BL:(c + 1) * BL]                      # [BL, DK, S]
        # k-major pieces for chunks 0..3: km[ci, k] = [P, SW]
        km = np.empty((NKM, KT, P, SW), dtype=np.float16)
        for ci in range(NKM):
            b_, sb_ = ci // 2, ci % 2
            for k in range(KT):
                km[ci, k] = shard[b_, k * P:(k + 1) * P,
                                  sb_ * SW:(sb_ + 1) * SW]
        ekm = km[0]                                            # [KT, P, SW]
        # all 4 chunks as k-pair pieces [P, 2*SW]
        ekmp = np.concatenate(
            [km[:, 0::2], km[:, 1::2]], axis=3)                # [4, KT/2, P, 2SW]
        # d-major chunks 4..15: enc[j] = [P, KT*SW] k-major columns
        sh5 = shard.reshape(BL, KT, P, SBLK, SW)               # [b, kt, p, sb, s]
        sh5 = np.ascontiguousarray(np.transpose(sh5, (0, 3, 2, 1, 4)))
        ench = sh5.reshape(NCH, P, KT * SW)[NKM:]
        hp = hpre[c * BL:(c + 1) * BL]                         # [BL, DD]
        sm = np.empty((P, SMC), dtype=np.float32)
        for d in range(DT):
            sm[:, d * BL:(d + 1) * BL] = hp[:, d * P:(d + 1) * P].T
            sm[:, DT * BL + d] = vr[d]
        sm[:, SMC - 1] = 1.0
        in_maps.append({
            "ekm": np.ascontiguousarray(ekm),
            "ekmp": np.ascontiguousarray(ekmp),
            "enc": np.ascontiguousarray(ench),
            "wek": wek,
            "sm": np.ascontiguousarray(sm),
        })

    return run_bass_kernel_spmd(
        nc, in_maps, core_ids=list(range(NCORES)), trace=trace, tmpdir=tmpdir
    )


def kernel(hidden, encoder_outputs, W, b, v):
    res = _run_spmd(hidden, encoder_outputs, W, b, v)
    out = np.concatenate([res.results[c]["out"] for c in range(NCORES)], axis=0)
    return out.astype(np.float32)


def run_traced(hidden, encoder_outputs, W, b, v):
    return _run_spmd(hidden, encoder_outputs, W, b, v, trace=True)


# revision 37
# speedup vs baseline: 1.0174x; 1.0174x over previous
"""Trainium2 Bass kernel for nn_Attention_50027779064227.

Computes softmax(v . tanh([hidden, enc] @ W + b)) over the source axis.
Data-parallel over batch across 8 NeuronCores; W/b/v replicated.

Algebraic split: concat([hid, enc]) @ W = hidden @ W_h (tiny -> computed
on HOST, shipped as a 16KB per-partition bias table) + enc @ W_e (the
big matmul, fp16 operands at full TensorE rate, fp32 PSUM accumulation).
The host-side h-part plus bias b is folded into the ScalarE tanh
activation as a per-partition bias. The v-dot (cross-partition
reduction) is an INCREMENTAL VectorE fold (one step per tanh d-block as
it lands) plus one ones-vector matmul per chunk; per-batch softmax runs
inline, with exp reading the score rows directly from PSUM (no DVE
copy). No max-subtraction: |scores| < 30 here, fp32 exp is safe.

Startup choreography (the kernel is PE-stream-bound; the binding
constraints measured on HW: ~0.65us HWDGE ring emission per dma_start,
8 global completion-sem lanes pacing ~8 transfers in flight, equal-share
packet round-robin across everything queued, and a ~3.4us PE-idle
window that drops the clock from 2.4 to 1.2GHz):
- the first FOUR chunks are processed K-MAJOR; chunk0 and the k-major
  weights ([KT, P, DT*P]) arrive as 128KB k-granular pieces at the
  head of the two HWDGE rings (first real matmul needs one piece from
  each ring); chunks 1-3 follow as 256KB k-pairs, chunk1 split across
  both rings; the small tables ride as ONE merged DMA behind chunk0;
- steady chunks 4-15 are d-major from whole-1MB prefetches, 3 deep
  (a 1MB chunk's round-robin share of the congested pipe is ~70GB/s);
- a warmup burst (M=1 matmuls into the score PSUM bank) spans the
  startup DMA window so the HAM clock-gate opens to 2.4GHz before real
  work; sized to worst-case data arrival because a too-short burst
  costs a ~2.5us cold-clock restart, a too-long one <=1us of queueing.
"""
import sys

for _p in ("/opt/trn_rl_repo",):
    if _p not in sys.path:
        sys.path.insert(0, _p)

import numpy as np
import concourse.bass as bass
import concourse.bacc as bacc
import concourse.mybir as mybir
from concourse.tile import TileContext
from concourse.bass_utils import run_bass_kernel_spmd

P = 128
NCORES = 8
B, S, DK, DD = 64, 1024, 1024, 512  # batch, src len, 2*ENC_HID, DEC_HID
BL = B // NCORES                    # 8 batches per core
SW = 512                            # moving-dim tile (s columns per matmul)
SBLK = S // SW                      # 2 s-blocks
KT = DK // P                        # 8 k-tiles for W_e
DT = DD // P                        # 4 d-blocks
NKM = 4                             # chunks processed k-major at startup
NCH = BL * SBLK                     # 16 chunks per core
NWARM = 15
SMC = DT * BL + DT + 1              # smalls cols: hpre | v | ones

F32 = mybir.dt.float32
F32R = mybir.dt.float32r
F16 = mybir.dt.float16
TANH = mybir.ActivationFunctionType.Tanh
EXP = mybir.ActivationFunctionType.Exp
IDENT = mybir.ActivationFunctionType.Identity

_BUILT = None


def _build():
    nc = bacc.Bacc()
    # chunk0/weights arrive as 128KB k-singles (k-pair experiments put too
    # many bytes in flight and delayed the FIRST piece by ~3us -> HAM cold
    # restart); chunks 1-3 as 256KB k-pairs
    ekm_d = nc.declare_dram_parameter("ekm", [KT, P, SW], F16, isOutput=False)
    ekmp_d = nc.declare_dram_parameter("ekmp", [NKM, KT // 2, P, 2 * SW],
                                       F16, isOutput=False)
    # whole-chunk tiles for the d-major steady chunks 4..15
    enc_d = nc.declare_dram_parameter("enc", [NCH - NKM, P, KT * SW], F16,
                                      isOutput=False)
    wek_d = nc.declare_dram_parameter("wek", [KT, P, DT * P], F16, isOutput=False)
    sm_d = nc.declare_dram_parameter("sm", [P, SMC], F32, isOutput=False)
    out_d = nc.declare_dram_parameter("out", [BL, S], F32, isOutput=True)

    with TileContext(nc) as tc:
        with (
            tc.tile_pool(name="const", bufs=1) as cpool,
            tc.tile_pool(name="chunk", bufs=4) as chpool,
            tc.tile_pool(name="tanh", bufs=12) as thpool,
            tc.tile_pool(name="ps_e", bufs=6, space="PSUM") as pe_pool,
            tc.tile_pool(name="ps_sc", bufs=2, space="PSUM") as sc_pool,
        ):
            # --- HAM warmup: M=1 dummy matmuls (into the score PSUM bank,
            # costing no extra banks) keep the PE busy through the startup
            # DMA window so the clock-gate opens to 2.4GHz early ---
            warm = cpool.tile([P, 256], F16, tag="warm")
            nc.vector.memset(warm[:], 0.25)
            wps = sc_pool.tile([1, 256], F32, tag="scp", name="warmps")
            for i in range(NWARM):
                nc.tensor.matmul(wps[:], warm[:, 0:1], warm[:],
                                 start=(i == 0), stop=(i == NWARM - 1))

            # --- startup DMAs. HWDGE ring emission costs ~0.65us PER
            # dma_start regardless of size, so the critical prefix is
            # chunk0's k-granular pieces (Act) and the weight pieces (SP)
            # ONLY; the small tables ride as one merged DMA after chunk0,
            # and chunks 1-3 use k-pair pieces to halve emission count ---
            wek_t = cpool.tile([P, KT * DT * P], F16, tag="wek")
            ekm_t = [cpool.tile([P, KT * SW], F16, tag=f"ekm{c}", name=f"ekm{c}")
                     for c in range(NKM)]
            sm = cpool.tile([P, SMC], F32, tag="sm")

            # (a third SWDGE ring was tried here and LOST ~8us: its packets
            # round-robin against the latency-critical chunk0/weight pieces
            # with no QoS, starving the PE and re-throttling the HAM clock)
            DP = DT * P
            for k in range(KT):
                nc.sync.dma_start(wek_t[:, k * DP:(k + 1) * DP], wek_d[k])
                nc.scalar.dma_start(ekm_t[0][:, k * SW:(k + 1) * SW], ekm_d[k])
            # chunk1 k-pairs split across BOTH rings (behind 8 weks on SP
            # alone they landed ~2us late); then smalls, chunk2 (Act),
            # chunk3 (SP)
            for j in range(KT // 2):
                eng = nc.sync if j % 2 == 0 else nc.scalar
                eng.dma_start(ekm_t[1][:, 2 * j * SW:(2 * j + 2) * SW],
                              ekmp_d[1, j])
            nc.scalar.dma_start(sm[:], sm_d[:])
            for j in range(KT // 2):
                nc.scalar.dma_start(ekm_t[2][:, 2 * j * SW:(2 * j + 2) * SW],
                                    ekmp_d[2, j])
            for j in range(KT // 2):
                nc.sync.dma_start(ekm_t[3][:, 2 * j * SW:(2 * j + 2) * SW],
                                  ekmp_d[3, j])

            ones_f = cpool.tile([P, 1], F32, tag="onesf")
            nc.vector.memset(ones_f[:], 1.0)
            ones_c = cpool.tile([P, 1], F32R, tag="ones")
            nc.vector.tensor_copy(ones_c[:], ones_f[:])
            ones_t = ones_c[:]

            def we_ap(k, d):
                return wek_t[:, k * DT * P + d * P: k * DT * P + (d + 1) * P]

            def hpre_ap(d, b):
                return sm[:, d * BL + b:d * BL + b + 1]

            v_sc = [sm[:, DT * BL + d:DT * BL + d + 1] for d in range(DT)]

            # chunk ci = (batch, sblock), batch-major
            chunks = [(b, sb) for b in range(BL) for sb in range(SBLK)]
            pre_ch = {}

            def emit_chunk_dma(ci):
                t = chpool.tile([P, KT * SW], F16, tag="chunk", name=f"ch{ci}")
                eng = nc.scalar if ci % 2 == 0 else nc.sync
                eng.dma_start(t[:], enc_d[ci - NKM])
                pre_ch[ci] = t

            # --- per-chunk state ---
            u_tiles = {}        # ci -> fold accumulator (ready after d3 step)
            sc_tiles = {}       # ci -> score PSUM tile [1, SW]
            ex_tiles = {}       # batch -> exp row [1, S]
            ss_tiles = {}       # batch -> running sum [1, 1]

            def emit_tanh_fold(ci, d, pe, sl=None):
                """tanh of PSUM d-block into SBUF, then one incremental
                DVE fold step into u[ci]. `sl` is the column sub-slice of
                the s-window (None = full width)."""
                b = chunks[ci][0]
                if sl is None:
                    sl = slice(0, SW)
                th = thpool.tile([P, sl.stop - sl.start], F32R,
                                 tag="tanh", name=f"th{ci}d{d}")
                nc.scalar.activation(th[:], pe, TANH, bias=hpre_ap(d, b))
                if d == 0 and sl.stop - sl.start == SW:
                    u = thpool.tile([P, SW], F32R, tag="u", name=f"u{ci}")
                    u_tiles[ci] = u
                    nc.vector.tensor_scalar_mul(u[:], th[:], v_sc[0])
                else:
                    u = u_tiles[ci]
                    nc.vector.scalar_tensor_tensor(
                        u[:, sl], th[:], v_sc[d], u[:, sl],
                        op0=mybir.AluOpType.mult, op1=mybir.AluOpType.add,
                    )

            def emit_score(ci):
                """ones-vector matmul folding partitions of u[ci] into a
                score row kept in PSUM (exp reads it there directly)."""
                scp = sc_pool.tile([1, SW], F32, tag="scp", name=f"sc{ci}")
                nc.tensor.matmul(scp[:], ones_t, u_tiles[ci][:],
                                 start=True, stop=True)
                sc_tiles[ci] = scp

            def emit_exps(b):
                """Both sblock scores of batch b are in PSUM: exp them into
                the row tile with accumulated sums."""
                ex = cpool.tile([1, S], F32, tag=f"ex{b}", name=f"ex{b}")
                s0 = cpool.tile([1, 1], F32, tag=f"ss{b}a", name=f"ss{b}a")
                s1 = cpool.tile([1, 1], F32, tag=f"ss{b}b", name=f"ss{b}b")
                nc.scalar.activation(ex[:, 0:SW], sc_tiles.pop(2 * b)[:],
                                     EXP, accum_out=s0[:])
                nc.scalar.activation(ex[:, SW:S], sc_tiles.pop(2 * b + 1)[:],
                                     EXP, accum_out=s1[:])
                nc.vector.tensor_add(s0[:], s0[:], s1[:])
                ex_tiles[b] = ex
                ss_tiles[b] = s0

            def emit_row_softmax(b):
                # scale on ScalarE so the DVE stays free for fold steps
                rc = cpool.tile([1, 1], F32, tag=f"rc{b}", name=f"rc{b}")
                nc.vector.reciprocal(rc[:], ss_tiles[b][:])
                ex = ex_tiles[b]
                nc.scalar.activation(ex[:], ex[:], IDENT, scale=rc[:])
                nc.sync.dma_start(out_d[b:b + 1, :], ex[:])

            # --- k-major startup chunks: each matmul needs exactly one
            # 128KB weight piece + one 128KB enc piece ---
            for ci in range(NKM):
                pes = [pe_pool.tile([P, SW], F32, tag="pe", name=f"pe{ci}d{d}")
                       for d in range(DT)]
                for k in range(KT):
                    src = ekm_t[ci][:, k * SW:(k + 1) * SW]
                    for d in range(DT):
                        nc.tensor.matmul(pes[d][:], we_ap(k, d), src,
                                         start=(k == 0), stop=(k == KT - 1))
                    if k == 2 and ci >= 1:
                        emit_score(ci - 1)
                    if k == 4 and ci >= 2 and ci % 2 == 0:
                        emit_exps(ci // 2 - 1)
                    if k == 6 and ci % 2 == 0 and ci >= 2:
                        emit_row_softmax(ci // 2 - 1)
                for d in range(DT):
                    emit_tanh_fold(ci, d, pes[d][:])

            # prefetch first d-major chunks while chunk2/3 mains run
            # (depth 3: a 1MB chunk's round-robin share of the congested
            # SDMA pipe is only ~70GB/s, so 2-deep arrived ~0.7us late)
            emit_chunk_dma(NKM)
            emit_chunk_dma(NKM + 1)
            emit_chunk_dma(NKM + 2)

            # --- steady d-major chunks ---
            for ci in range(NKM, NCH):
                b, sb = chunks[ci]
                ch = pre_ch.pop(ci)
                if ci + 3 < NCH:
                    emit_chunk_dma(ci + 3)
                last = ci == NCH - 1

                def d_mains(d, pe, cols=slice(0, SW)):
                    n = cols.stop - cols.start
                    for k in range(KT):
                        nc.tensor.matmul(
                            pe[:, 0:n], we_ap(k, d),
                            ch[:, k * SW + cols.start: k * SW + cols.stop],
                            start=(k == 0), stop=(k == KT - 1))

                if not last:
                    for d in range(DT):
                        pe = pe_pool.tile([P, SW], F32, tag="pe", name="pe")
                        d_mains(d, pe)
                        emit_tanh_fold(ci, d, pe[:])
                        if d == 0:
                            emit_score(ci - 1)
                        if d == 1 and ci % 2 == 0:
                            emit_exps(ci // 2 - 1)
                        if d == 2 and ci % 2 == 0:
                            emit_row_softmax(ci // 2 - 1)
                else:
                    # final chunk (b7, s1): d0-2 normal; d3 in column halves
                    # with separate PSUM tiles so each half's tanh/fold/score
                    # chain starts at its own stop
                    for d in range(DT - 1):
                        pe = pe_pool.tile([P, SW], F32, tag="pe", name="pe")
                        d_mains(d, pe)
                        emit_tanh_fold(ci, d, pe[:])
                        if d == 0:
                            emit_score(ci - 1)      # (b7, s0)
                        if d == 1:
                            # early exp of (b7, s0) straight from PSUM
                            exL = cpool.tile([1, S], F32, tag="exL", name="exL")
                            s0L = cpool.tile([1, 1], F32, tag="s0L", name="s0L")
                            nc.scalar.activation(exL[:, 0:SW],
                                                 sc_tiles.pop(2 * b)[:],
                                                 EXP, accum_out=s0L[:])
                    peh = [pe_pool.tile([P, SW], F32, tag="pe", name=f"peh{h}")
                           for h in (0, 1)]
                    for h in (0, 1):
                        d_mains(DT - 1, peh[h], slice(h * 256, (h + 1) * 256))
                    # half 0's chain hides under half 1's mains; finer splits
                    # LOSE here: each extra piece adds a serialized
                    # exp+ACCUM_READ (~550ns) on ScalarE's strict FIFO
                    sadd = []
                    for h in (0, 1):
                        sl = slice(h * 256, (h + 1) * 256)
                        emit_tanh_fold(ci, DT - 1, peh[h][:, 0:256], sl=sl)
                        scp = sc_pool.tile([1, 256], F32, tag="scp",
                                           name=f"scL{h}")
                        nc.tensor.matmul(scp[:], ones_t, u_tiles[ci][:, sl],
                                         start=True, stop=True)
                        sh = cpool.tile([1, 1], F32, tag=f"sL{h}", name=f"sL{h}")
                        nc.scalar.activation(exL[:, SW + sl.start:SW + sl.stop],
                                             scp[:], EXP, accum_out=sh[:])
                        sadd.append(sh)
                    nc.vector.tensor_add(s0L[:], s0L[:], sadd[0][:])
                    nc.vector.tensor_add(s0L[:], s0L[:], sadd[1][:])
                    rc = cpool.tile([1, 1], F32, tag="rcL", name="rcL")
                    nc.vector.reciprocal(rc[:], s0L[:])
                    nc.vector.tensor_scalar_mul(exL[:, 0:SW], exL[:, 0:SW],
                                                rc[:])
                    nc.scalar.activation(exL[:, SW:S], exL[:, SW:S],
                                         IDENT, scale=rc[:])
                    nc.sync.dma_start(out_d[b:b + 1, :], exL[:])

    nc.finalize()
    return nc


def _run_spmd(hidden, encoder_outputs, W, b, v, trace=False, tmpdir=None):
    global _BUILT
    if _BUILT is None:
        _BUILT = _build()
    nc = _BUILT

    hidden = np.asarray(hidden, dtype=np.float64)
    W64 = np.asarray(W, dtype=np.float64)
    bv = np.asarray(b, dtype=np.float64)
    vv = np.asarray(v, dtype=np.float32)

    # weights fully k-major: wek[k] = W_e rows [k*128, (k+1)*128)
    wek = np.ascontiguousarray(
        np.asarray(W, dtype=np.float32)[DD:].astype(np.float16)
    ).reshape(KT, P, DT * P)

    # host-side tiny part: hpre[b] = hidden[b] @ W_h + b  -> [B, DD]
    hpre = (hidden @ W64[:DD] + bv).astype(np.float32)

    encT = np.transpose(np.asarray(encoder_outputs, dtype=np.float32),
                        (1, 2, 0)).astype(np.float16)     # [B, DK, S]
    vr = vv.reshape(DT, P)

    in_maps = []
    for c in range(NCORES):
        shard = encT[c * BL:(c + 1) * BL]                      # [BL, DK, S]
        # k-major pieces for chunks 0..3: km[ci, k] = [P, SW]
        km = np.empty((NKM, KT, P, SW), dtype=np.float16)
        for ci in range(NKM):
            b_, sb_ = ci // 2, ci % 2
            for k in range(KT):
                km[ci, k] = shard[b_, k * P:(k + 1) * P,
                                  sb_ * SW:(sb_ + 1) * SW]
        ekm = km[0]                                            # [KT, P, SW]
        # all 4 chunks as k-pair pieces [P, 2*SW]
        ekmp = np.concatenate(
            [km[:, 0::2], km[:, 1::2]], axis=3)                # [4, KT/2, P, 2SW]
        # d-major chunks 4..15: enc[j] = [P, KT*SW] k-major columns
        sh5 = shard.reshape(BL, KT, P, SBLK, SW)               # [b, kt, p, sb, s]
        sh5 = np.ascontiguousarray(np.transpose(sh5, (0, 3, 2, 1, 4)))
        ench = sh5.reshape(NCH, P, KT * SW)[NKM:]
        hp = hpre[c * BL:(c + 1) * BL]                         # [BL, DD]
        sm = np.empty((P, SMC), dtype=np.float32)
        for d in range(DT):
            sm[:, d * BL:(d + 1) * BL] = hp[:, d * P:(d + 1) * P].T
            sm[:, DT * BL + d] = vr[d]
        sm[:, SMC - 1] = 1.0
        in_maps.append({
            "ekm": np.ascontiguousarray(ekm),
            "ekmp": np.ascontiguousarray(ekmp),
            "enc": np.ascontiguousarray(ench),
            "wek": wek,
            "sm": np.ascontiguousarray(sm),
        })

    return run_bass_kernel_spmd(
        nc, in_maps, core_ids=list(range(NCORES)), trace=trace, tmpdir=tmpdir
    )


def kernel(hidden, encoder_outputs, W, b, v):
    res = _run_spmd(hidden, encoder_outputs, W, b, v)
    out = np.concatenate([res.results[c]["out"] for c in range(NCORES)], axis=0)
    return out.astype(np.float32)


def run_traced(hidden, encoder_outputs, W, b, v):
    return _run_spmd(hidden, encoder_outputs, W, b, v, trace=True)
